# revision 1
# baseline (speedup 1.0000x reference)
"""Bilateral blur (7x7, L1 color distance) on 8 Trainium2 NeuronCores.

Input (4, 3, 512, 512) fp32 -> output (4, 3, 512, 512) fp32.
Sharding: pure data parallelism - core i handles batch i//2, row-half i%2
(256 output rows x 512 cols). The host reflect-pads each image and re-lays
each shard into a "transposed-blocked" layout: partition p (0..127) owns
output columns [4p, 4p+4); its free dim holds, per channel, a 10x262 patch
(padded cols 4p..4p+9 x 262 padded rows, flattened xl*262+y), so all 49 tap
shifts (dy, dx) become pure free-dim AP offsets. Two fp16 copies (xe, and
xo shifted one row) keep every hot DVE access-pattern 4-byte aligned, the
DVE fp16 2x-mode requirement.

Key optimizations over the straightforward per-tap pipeline:

1. One-activation Gaussian: erf'(x) = (2/sqrt(pi)) exp(-x^2), so the color
   weight u = exp(-50 d^2) is a single Derivative_Erf (scale=sqrt(50))
   instead of Square+Exp. The space-kernel factor s_k*sqrt(pi)/2 is folded
   into per-tap SCALED IDENTITY matmul stationaries (10 distinct values by
   kernel symmetry).

2. Mirror-symmetry weight reuse: u_k(p) = u_{48-k}(p + d_k), so the whole
   distance pipeline (sub, |.|, channel-sum) + Derivative_Erf runs only for
   the 24 lower-half taps + never for the center, each on an extended
   domain D_k = O u (O - d_k) (4+|dx| cols x 256+|dy| rows ~ 0.73x the
   two-tap work). The mirrored tap reads u_k at AP offset -d_k; for odd-dy
   pairs an ACT Copy re-aligns a 4x256 window of u to an even offset for
   the DVE multiply (Pool/PE readers don't need alignment).

3. The center tap (u == 1) costs nothing: its 8 matmuls stream the input
   windows (and a ones tile for the denominator column) straight from
   SBUF. Every tap's denominator column is likewise read directly from the
   u buffer by the PE - w is never copied into the product tile.

4. PSUM-bank-decoupled accumulation: each of the 8 banks is an independent
   accumulation stream (start/stop are per-bank), so Pool-fed product
   banks are emitted one pair later than DVE-fed ones and Pool latency
   never stalls the PE. The bulk stage (multiplies + matmuls) of each pair
   is emitted one pair behind the distance pipeline (software pipelining
   of the in-order engine queues).

5. Engine balance (cost-model LP): DVE does sub + both chain-adds + 2 mult
   channels (fp16 TT 2x); ACT does |.| + Derivative_Erf + parity copies;
   Pool does the 3rd mult channel (2 channels on 12 of 48 taps); PE does
   the scaled-identity accumulate. Tail reads PSUM directly.
"""
import numpy as np

import concourse.bass as bass
import concourse.bacc as bacc
import concourse.mybir as mybir
from concourse.tile import TileContext
from concourse import bass_utils

C = 3
B, H, W = 4, 512, 512
KX = KY = 7
PAD = 3
SIGMA_COLOR = 0.1
N_CORES = 8

ROWS = 256
WG = 4
NPART = 128
XE, YE = WG + 2 * PAD, ROWS + 2 * PAD
FREE_IN = XE * YE
FREE_OUT = WG * ROWS
NTAP = KY * KX

XWMAX = WG + PAD            # 7
YWPMAX = ROWS + PAD + 1     # 260
EWMAX = XWMAX * YWPMAX      # 1820


def _space_kernel():
    def g1(k, sigma):
        x = np.arange(k, dtype=np.float64) - (k - 1) / 2.0
        g = np.exp(-0.5 * (x / sigma) ** 2)
        return g / g.sum()
    gy, gx = g1(KY, 1.5), g1(KX, 1.5)
    return (gy[:, None] * gx[None, :]).reshape(-1)


def _sidt_table():
    sk = _space_kernel().reshape(KY, KX) * (np.sqrt(np.pi) / 2.0)
    buckets = {}
    k2b = np.zeros(NTAP, np.int32)
    for dy in range(KY):
        for dx in range(KX):
            iy, ix = min(dy, 6 - dy), min(dx, 6 - dx)
            key = (min(iy, ix), max(iy, ix))
            if key not in buckets:
                buckets[key] = (len(buckets), sk[dy, dx])
            k2b[dy * KX + dx] = buckets[key][0]
    vals = np.array([v for _, v in sorted(buckets.values())], np.float64)
    return vals, k2b


_SVALS, _K2B = _sidt_table()
NBKT = len(_SVALS)


def _tap_ap(t, dx, dy, nch=C, ch0=0, dy_base=0):
    a = t[:]
    return bass.AP(a.tensor, a.offset + ch0 * FREE_IN + dx * YE + (dy - dy_base),
                   [[C * FREE_IN, NPART], [FREE_IN, nch], [YE, WG], [1, ROWS]])


def _stk_ap(t, nch=C, ch0=0, step0=False):
    a = t[:]
    tot = a.shape[1]
    return bass.AP(a.tensor, a.offset + ch0 * FREE_OUT,
                   [[tot, NPART], [0 if step0 else FREE_OUT, nch], [ROWS, WG], [1, ROWS]])


def _ext_ap(t, xw, yw, ywp, nch=1, off=0):
    """Ragged AP over an extended-domain buffer: xw col-blocks of stride ywp,
    yw live rows each; optional channel dim of stride EWMAX."""
    a = t[:]
    dims = [[a.shape[1], NPART]]
    if nch > 1:
        dims.append([EWMAX, nch])
    dims += [[ywp, xw], [1, yw]]
    return bass.AP(a.tensor, a.offset + off, dims)


def _build(ntaps=NTAP):
    """ntaps: 49 = full kernel; smaller values emit the center + the first
    (ntaps-1)//2 mirror pairs (used by the delta timer)."""
    nc = bacc.Bacc()
    f32 = mybir.dt.float32
    f16 = mybir.dt.float16
    xe = nc.dram_tensor("xe", [NPART, C * FREE_IN], f16, kind="ExternalInput")
    xo = nc.dram_tensor("xo", [NPART, C * FREE_IN], f16, kind="ExternalInput")
    sidt = nc.dram_tensor("sidt", [NPART, 2 * NBKT * NPART], f16, kind="ExternalInput")
    o = nc.dram_tensor("o", [NPART, C * FREE_OUT], f16, kind="ExternalOutput")
    AOT = mybir.AluOpType
    AFT = mybir.ActivationFunctionType
    F = FREE_OUT
    SQ50 = float(np.sqrt(0.5) / SIGMA_COLOR)
    MN = 512
    NPAIR = min(24, max(1, (ntaps - 1) // 2)) if ntaps > 1 else 0
    q2_pool = set()


    # big extended-domains first (ramp does only sub/abs), smallest last
    # (the drain runs a full serial chain over the final pair)
    order = [7, 0, 6, 14, 1, 8, 20, 2, 15, 9, 21, 3, 16, 11, 4, 17, 12, 5,
             18, 19, 13, 10, 22, 23][:NPAIR]

    with TileContext(nc) as tc:
        with tc.tile_pool(name="persist", bufs=1) as pool, \
             tc.tile_pool(name="big", bufs=4) as bp, \
             tc.tile_pool(name="dltp", bufs=4) as dp, \
             tc.tile_pool(name="adlp", bufs=3) as ap_, \
             tc.tile_pool(name="sml", bufs=3) as sp, \
             tc.tile_pool(name="ps", bufs=1, space="PSUM") as psp:
            To = pool.tile([NPART, C * FREE_IN], f16, name="To")
            nc.sync.dma_start(To[:], xo[:])
            Te = pool.tile([NPART, C * FREE_IN], f16, name="Te")
            nc.sync.dma_start(Te[:], xe[:])
            sid = pool.tile([NPART, 2 * NBKT * NPART], f16, name="sid")
            nc.sync.dma_start(sid[:], sidt[:])
            ones = pool.tile([NPART, MN], f16, name="ones")
            nc.vector.memset(ones[:], 1.0)

            def tile_for(dy):
                return (Te, 0) if dy % 2 == 0 else (To, 1)

            acc = psp.tile([NPART, 4 * F], f32, name="acc")
            started = set()

            def mm(g, bslot, mov, last):
                first = g not in started
                started.add(g)
                nc.tensor.matmul(acc[:, g * MN:(g + 1) * MN],
                                 sid[:, bslot * NPART:(bslot + 1) * NPART],
                                 mov, start=first, stop=last)

            # ---- center tap (u == 1, dlt == 0): den banks only ----
            mm(6, int(_K2B[24]), ones[:], ntaps == 1)
            mm(7, int(_K2B[24]), ones[:], ntaps == 1)

            # ---- 24 mirror pairs ----
            pending = []      # lagged PE flushes (main banks)
            pending_q = []    # lagged Q-mult emissions
            pending_c = []    # lagged chain tails (adds + derf)
            pending_l = []    # doubly-lagged Pool-fed bank flushes
            for pi in range(NPAIR):
                k = order[pi]
                dy, dx = k // KX, k % KX
                dys, dxs = dy - PAD, dx - PAD
                x0 = min(0, -dxs)
                xw = WG + abs(dxs)
                yw = ROWS + abs(dys)
                ywp = yw + (yw & 1)

                tt, tb = tile_for(dy)
                dlt = dp.tile([NPART, C * EWMAX], f16, name="dlt", tag="dlt")
                nc.vector.tensor_tensor(
                    out=_ext_ap(dlt, xw, yw, ywp, nch=C),
                    in0=bass.AP(tt[:].tensor,
                                tt[:].offset + (x0 + PAD + dxs) * YE + (PAD + dys - tb),
                                [[C * FREE_IN, NPART], [FREE_IN, C], [YE, xw], [1, yw]]),
                    in1=bass.AP(To[:].tensor,
                                To[:].offset + (x0 + PAD) * YE + (PAD - 1),
                                [[C * FREE_IN, NPART], [FREE_IN, C], [YE, xw], [1, yw]]),
                    op=AOT.subtract)
                adl = ap_.tile([NPART, C * EWMAX], f16, name="adl", tag="adl")
                nc.scalar.activation(_ext_ap(adl, xw, yw, ywp, nch=C),
                                     _ext_ap(dlt, xw, yw, ywp, nch=C),
                                     AFT.Abs, bias=0.0, scale=1.0)
                if len(pending_c) > 1:
                    pending_c.pop(0)()
                if len(pending_q) > 1:
                    pending_q.pop(0)()
                U = sp.tile([NPART, EWMAX], f16, name="U", tag="U")

                def emit_chain_tail(_adl=adl, _U=U, _xw=xw, _yw=yw, _ywp=ywp):
                    s01 = sp.tile([NPART, EWMAX], f16, name="s01", tag="s01")
                    nc.vector.tensor_tensor(
                        out=_ext_ap(s01, _xw, _yw, _ywp),
                        in0=_ext_ap(_adl, _xw, _yw, _ywp),
                        in1=_ext_ap(_adl, _xw, _yw, _ywp, off=EWMAX),
                        op=AOT.add)
                    dsum = sp.tile([NPART, EWMAX], f16, name="dsum", tag="dsum")
                    nc.vector.tensor_tensor(
                        out=_ext_ap(dsum, _xw, _yw, _ywp),
                        in0=_ext_ap(s01, _xw, _yw, _ywp),
                        in1=_ext_ap(_adl, _xw, _yw, _ywp, off=2 * EWMAX),
                        op=AOT.add)
                    nc.scalar.activation(_ext_ap(_U, _xw, _yw, _ywp),
                                         _ext_ap(dsum, _xw, _yw, _ywp),
                                         AFT.Derivative_Erf, bias=0.0, scale=SQ50)
                pending_c.append(emit_chain_tail)

                # Q = u (x) dlt on the full extended domain: the direct tap's
                # num contribution is +s.Q at the direct offset, the mirror
                # tap's is -s.Q at the mirror offset (dlt_mirror(p) =
                # -dlt(p-d), u_mirror(p) = u(p-d)). Emission is lagged one
                # pair (flushed after the next pair's sub) so DVE chews it
                # while ACT runs |.| instead of stalling behind derf.
                npool = 2 if pi in q2_pool else 1
                nd = 3 - npool
                Q = bp.tile([NPART, C * EWMAX], f16, name="Q", tag="Q")

                def emit_qmult(_Q=Q, _dlt=dlt, _U=U, _xw=xw, _yw=yw,
                               _ywp=ywp, _nd=nd, _np=npool):
                    nc.vector.tensor_tensor(
                        out=_ext_ap(_Q, _xw, _yw, _ywp, nch=_nd),
                        in0=_ext_ap(_dlt, _xw, _yw, _ywp, nch=_nd),
                        in1=bass.AP(_U[:].tensor, _U[:].offset,
                                    [[EWMAX, NPART], [0, _nd], [_ywp, _xw], [1, _yw]]),
                        op=AOT.mult)
                    nc.gpsimd.tensor_tensor(
                        out=_ext_ap(_Q, _xw, _yw, _ywp, nch=_np, off=_nd * EWMAX),
                        in0=_ext_ap(_dlt, _xw, _yw, _ywp, nch=_np, off=_nd * EWMAX),
                        in1=bass.AP(_U[:].tensor, _U[:].offset,
                                    [[EWMAX, NPART], [0, _np], [_ywp, _xw], [1, _yw]]),
                        op=AOT.mult)
                pending_q.append(emit_qmult)

                bkt = int(_K2B[k])
                db = (-x0) * ywp
                mb = (-dxs - x0) * ywp + abs(dys)

                def flush_main(last, _U=U, _Q=Q, _ywp=ywp, _bkt=bkt,
                               _db=db, _mb=mb, _nd=nd):
                    uv, qv = _U[:], _Q[:]
                    # +s: den (direct + mirror), then DVE-fed num banks direct
                    for base in (_db, _mb):
                        for xh in range(2):
                            mm(6 + xh, _bkt,
                               bass.AP(uv.tensor, uv.offset + base + 2 * xh * _ywp,
                                       [[EWMAX, NPART], [_ywp, 2], [1, ROWS]]),
                               last)
                    dve_banks = list(range(2 * _nd))
                    for g in dve_banks:
                        c, xh = g // 2, g % 2
                        mm(g, _bkt,
                           bass.AP(qv.tensor,
                                   qv.offset + c * EWMAX + _db + 2 * xh * _ywp,
                                   [[C * EWMAX, NPART], [_ywp, 2], [1, ROWS]]),
                           last)
                    # -s: DVE-fed num banks mirror
                    for g in dve_banks:
                        c, xh = g // 2, g % 2
                        mm(g, NBKT + _bkt,
                           bass.AP(qv.tensor,
                                   qv.offset + c * EWMAX + _mb + 2 * xh * _ywp,
                                   [[C * EWMAX, NPART], [_ywp, 2], [1, ROWS]]),
                           last)

                    def flush_pool_banks(last2):
                        for g in range(2 * _nd, 6):
                            c, xh = g // 2, g % 2
                            mm(g, _bkt,
                               bass.AP(qv.tensor,
                                       qv.offset + c * EWMAX + _db + 2 * xh * _ywp,
                                       [[C * EWMAX, NPART], [_ywp, 2], [1, ROWS]]),
                               last2)
                        for g in range(2 * _nd, 6):
                            c, xh = g // 2, g % 2
                            mm(g, NBKT + _bkt,
                               bass.AP(qv.tensor,
                                       qv.offset + c * EWMAX + _mb + 2 * xh * _ywp,
                                       [[C * EWMAX, NPART], [_ywp, 2], [1, ROWS]]),
                               last2)
                    return flush_pool_banks

                pending.append(flush_main)
                if len(pending) > 2:
                    pending_l.append(pending.pop(0)(False))
                    while len(pending_l) > 1:
                        pending_l.pop(0)(False)

            # drain
            while pending_c:
                pending_c.pop(0)()
            while pending_q:
                pending_q.pop(0)()
            while len(pending) > 1:
                pending_l.append(pending.pop(0)(False))
            if pending:
                pending_l.append(pending.pop(0)(True))
            while pending_l:
                pending_l.pop(0)(len(pending_l) == 1)

            # tail: out_c = x_c + numdelta_c * recip  (x = center pixel)
            recip = pool.tile([NPART, F], f32, name="recip")
            nc.vector.reciprocal(recip[:], acc[:, 3 * F:])
            ttmp = pool.tile([NPART, F], f32, name="ttmp")
            ot = pool.tile([NPART, C * F], f16, name="ot")
            for c in range(C):
                nc.vector.tensor_tensor(
                    out=bass.AP(ttmp[:].tensor, ttmp[:].offset,
                                [[F, NPART], [ROWS, WG], [1, ROWS]]),
                    in0=bass.AP(acc[:].tensor, acc[:].offset + c * F,
                                [[4 * F, NPART], [ROWS, WG], [1, ROWS]]),
                    in1=bass.AP(recip[:].tensor, recip[:].offset,
                                [[F, NPART], [ROWS, WG], [1, ROWS]]),
                    op=AOT.mult)
                nc.vector.tensor_tensor(
                    out=_stk_ap(ot, nch=1, ch0=c),
                    in0=bass.AP(ttmp[:].tensor, ttmp[:].offset,
                                [[F, NPART], [ROWS, WG], [1, ROWS]]),
                    in1=bass.AP(To[:].tensor,
                                To[:].offset + c * FREE_IN + PAD * YE + (PAD - 1),
                                [[C * FREE_IN, NPART], [YE, WG], [1, ROWS]]),
                    op=AOT.add)
                nc.sync.dma_start(o[:, c * F:(c + 1) * F], ot[:, c * F:(c + 1) * F])
    return nc


_COLIDX = np.arange(NPART)[:, None] * WG + np.arange(XE)[None, :]


def _shard_layout(shard, yshift):
    buf = np.zeros((NPART, C, XE, YE), np.float16)
    for c in range(C):
        blk = shard[c].T[_COLIDX]
        if yshift:
            buf[:, c, :, :YE - yshift] = blk[:, :, yshift:]
        else:
            buf[:, c] = blk
    return buf.reshape(NPART, C * FREE_IN)


def _sidt_payload():
    out = np.zeros((NPART, 2 * NBKT * NPART), np.float16)
    for b in range(NBKT):
        out[:, b * NPART:(b + 1) * NPART] = np.eye(NPART) * _SVALS[b]
        out[:, (NBKT + b) * NPART:(NBKT + b + 1) * NPART] = \
            np.eye(NPART) * -_SVALS[b]
    return out


_SIDT = _sidt_payload()

_NC_CACHE = {}


def _get_nc():
    if "nc" not in _NC_CACHE:
        nc = _build()
        nc.finalize()
        _NC_CACHE["nc"] = nc
    return _NC_CACHE["nc"]


def make_in_maps(x):
    xp = np.pad(x, ((0, 0), (0, 0), (PAD, PAD), (PAD, PAD)), mode="reflect")
    in_maps = []
    for core in range(N_CORES):
        b, half = core // 2, core % 2
        r0 = half * ROWS
        shard = xp[b, :, r0:r0 + ROWS + 2 * PAD, :]
        in_maps.append({"xe": _shard_layout(shard, 0),
                        "xo": _shard_layout(shard, 1),
                        "sidt": _SIDT})
    return in_maps


def kernel(input: np.ndarray) -> np.ndarray:
    x = np.asarray(input, dtype=np.float32)
    assert x.shape == (B, C, H, W)
    in_maps = make_in_maps(x)
    nc = _get_nc()
    res = bass_utils.run_bass_kernel_spmd(nc, in_maps, list(range(N_CORES)))
    out = np.empty((B, C, H, W), np.float32)
    for core in range(N_CORES):
        b, half = core // 2, core % 2
        r0 = half * ROWS
        ov = np.asarray(res.results[core]["o"]).reshape(NPART, C, WG, ROWS)
        for c in range(C):
            out[b, c, r0:r0 + ROWS, :] = ov[:, c].transpose(2, 0, 1).reshape(ROWS, W)
    return out

